# revision 13
# baseline (speedup 1.0000x reference)
"""DeepSet segment-reduce kernel for 8 Trainium2 NeuronCores.

Math (equivalent to the reference, using linearity of segment_sum):
    r      = relu(x @ W1 + b1)                      # per-node, on device
    sums_r = segment_sum(r)                         # [B, HID]
    mean_r = sums_r / max(counts, 1)                # counts via host bincount
    hid    = mean_r @ W2 + b2                       # tiny tail, on device
    out    = relu(hid @ W3 + b3) @ W4 + b4          # tiny tail, on device

Phase 1 (8 cores, data-parallel over nodes): each core's ~N/8 nodes are
split into two contiguous halves packed on SBUF partition halves (features
of half A in partitions 0..63, half B in 64..127, bf16), so DMA runs at
full 128-partition width.  Each half's segment runs are zero-padded to
multiples of 1024 columns, so every 1024-column superblock belongs to one
segment.  Matmuls use K=128 with zero-padded weights wzA=[[W1],[0]],
wzB=[[0],[W1]] — full-array matmuls keep the PE's HAM activity monitor
un-throttled at 2.4 GHz (K=64 matmuls leave it stuck at 1.2 GHz), and both
weight tiles live at PE tile position (0,0) (bf16 LDWEIGHTS at row tile 64
is broken in hardware).  Per superblock and half: 2 matmuls fill a 2-bank
PSUM tile, then ONE fused relu+bias+sum produces the [128,1] partial:
half A on the Scalar engine (ACT Relu, bias, accum_out), half B on the
Vector engine (tensor_scalar max(-b1)/add-reduce; the sum is off by
SB*b1, restored on the host).  With 4 PSUM tiles in flight both reduce
engines run continuously while the PE refills the other tiles.

The host routes superblock partials to segments, removes the pad columns'
relu(b1) contribution, applies the mean, and a second tiny NEFF runs the
rho MLP in bf16.  Segments with zero nodes are fixed up on the host
(reference gives relu(b3) @ W4 + b4 there).
"""

import os
import sys

for _p in ("/opt/trn_rl_repo",):
    if os.path.isdir(_p) and _p not in sys.path:
        sys.path.append(_p)

import numpy as np
import ml_dtypes

import concourse.bass as bass
import concourse.tile as tile
from concourse import bacc, mybir
from concourse.bass_utils import run_bass_kernel_spmd

F32 = mybir.dt.float32
BF16 = mybir.dt.bfloat16

NCORES = 8
TILE = 512
SB = 1024            # superblock columns (2 PSUM banks; 4 tiles in flight)
SB_PER_CHUNK = 8     # superblocks per DMA chunk
NSEG = 1024
ODIM = 16


def _pad_runs(ids, lo, ch):
    """Segment runs of a sorted id slice, padded to SB multiples.
    Returns (src indices with -1 pads, seg id per superblock)."""
    uniq, starts = np.unique(ids, return_index=True)
    ends = np.append(starts[1:], ch)
    seg_of_sb = []
    src_parts = []
    for k in range(len(uniq)):
        L = int(ends[k] - starts[k])
        T = -(-L // SB)
        arr = np.full(T * SB, -1, dtype=np.int64)
        arr[:L] = lo + starts[k] + np.arange(L)
        src_parts.append(arr)
        seg_of_sb += [int(uniq[k])] * T
    src = np.concatenate(src_parts) if src_parts else np.empty(0, np.int64)
    return src, seg_of_sb


def _host_prep(x, x_batch, ncores=NCORES):
    N = x.shape[0]
    assert N % (2 * ncores) == 0
    ch = N // (2 * ncores)          # nodes per half
    xb = np.asarray(x_batch)

    counts = np.bincount(xb, minlength=NSEG).astype(np.float64)

    halves = []                      # (src, seg_of_sb) per (core, half)
    n_sb = 0
    for c in range(ncores):
        for h in range(2):
            lo = (2 * c + h) * ch
            src, seg_of_sb = _pad_runs(xb[lo:lo + ch], lo, ch)
            halves.append((src, seg_of_sb))
            n_sb = max(n_sb, len(seg_of_sb))

    cols = n_sb * SB
    padcount = np.zeros(NSEG, dtype=np.float64)
    xts = []
    seg_a, seg_d = [], []
    for c in range(ncores):
        xt = np.zeros((128, cols), dtype=ml_dtypes.bfloat16)
        for h in range(2):
            src, seg_of_sb = halves[2 * c + h]
            if len(src) < cols:
                src = np.concatenate([src, np.full(cols - len(src), -1, np.int64)])
            mask = src >= 0
            gath = np.zeros((cols, 64), dtype=np.float32)
            gath[mask] = x[src[mask]]
            xt[64 * h:64 * h + 64, :] = gath.T.astype(ml_dtypes.bfloat16)
            if seg_of_sb:
                seg_arr = np.array(seg_of_sb, dtype=np.int64)
                real = mask[:len(seg_arr) * SB].reshape(-1, SB).sum(axis=1)
                np.add.at(padcount, seg_arr, SB - real)
            (seg_a if h == 0 else seg_d).append(seg_of_sb)
        xts.append(xt)

    meta = dict(n_sb=n_sb, cols=cols, counts=counts, padcount=padcount,
                seg_a=seg_a, seg_d=seg_d, ncores=ncores)
    return xts, meta


def _build_phase1(n_sb, cols, ncores=NCORES):
    nc = bacc.Bacc("TRN2", target_bir_lowering=False, debug=False,
                   num_devices=ncores)
    xt_d = nc.dram_tensor("xt", [128, cols], BF16, kind="ExternalInput").ap()
    wza_d = nc.dram_tensor("wza", [128, 128], BF16, kind="ExternalInput").ap()
    wzb_d = nc.dram_tensor("wzb", [128, 128], BF16, kind="ExternalInput").ap()
    b1_d = nc.dram_tensor("b1", [128, 1], F32, kind="ExternalInput").ap()
    nb1_d = nc.dram_tensor("nb1", [128, 1], F32, kind="ExternalInput").ap()
    sa_d = nc.dram_tensor("s_act", [128, n_sb], F32, kind="ExternalOutput").ap()
    sd_d = nc.dram_tensor("s_dve", [128, n_sb], F32, kind="ExternalOutput").ap()

    CH = SB_PER_CHUNK * SB

    with tile.TileContext(nc) as tc:
        with tc.tile_pool(name="const", bufs=1) as cpool, \
             tc.tile_pool(name="xin", bufs=3) as xpool, \
             tc.tile_pool(name="tr", bufs=1) as trpool, \
             tc.tile_pool(name="ps", bufs=2, space="PSUM") as pspool:

            wza = cpool.tile([128, 128], BF16)
            nc.sync.dma_start(wza[:], wza_d[:])
            wzb = cpool.tile([128, 128], BF16)
            nc.sync.dma_start(wzb[:], wzb_d[:])
            b1t = cpool.tile([128, 1], F32)
            nc.sync.dma_start(b1t[:], b1_d[:])
            nb1t = cpool.tile([128, 1], F32)
            nc.sync.dma_start(nb1t[:], nb1_d[:])
            S_a = cpool.tile([128, n_sb], F32)
            nc.vector.memset(S_a[:], 0.0)
            S_d = cpool.tile([128, n_sb], F32)
            nc.vector.memset(S_d[:], 0.0)

            xtile = None
            for sb in range(n_sb):
                if sb % SB_PER_CHUNK == 0:
                    # one SBUF chunk, filled by per-superblock DMAs so the
                    # first matmuls start after ~256 KB instead of ~2 MB
                    xtile = xpool.tile([128, CH], BF16, tag="x")
                    for j in range(min(SB_PER_CHUNK, n_sb - sb)):
                        lo = (sb + j) * SB
                        nc.sync.dma_start(xtile[:, j * SB:(j + 1) * SB],
                                          xt_d[:, lo:lo + SB])
                base = (sb % SB_PER_CHUNK) * SB
                psa = pspool.tile([128, SB], F32, tag="psa")
                psb = pspool.tile([128, SB], F32, tag="psb")
                for t in range(SB // TILE):
                    off = base + t * TILE
                    nc.tensor.matmul(
                        psb[:, t * TILE:t * TILE + TILE], lhsT=wzb[:],
                        rhs=xtile[:, off:off + TILE], start=True, stop=True)
                    nc.tensor.matmul(
                        psa[:, t * TILE:t * TILE + TILE], lhsT=wza[:],
                        rhs=xtile[:, off:off + TILE], start=True, stop=True)
                trash_a = trpool.tile([128, SB], BF16, tag="ta")
                nc.scalar.activation(
                    out=trash_a[:], in_=psa[:],
                    func=mybir.ActivationFunctionType.Relu,
                    bias=b1t[:, 0:1],
                    accum_out=S_a[:, sb:sb + 1])
                # accum_out = add-reduce of max(psum, -b1)
                #           = sum(relu(psum + b1)) - SB*b1  (host adds it back)
                trash_d = trpool.tile([128, SB], BF16, tag="td")
                nc.vector.tensor_scalar(
                    out=trash_d[:], in0=psb[:],
                    scalar1=nb1t[:, 0:1], scalar2=0.0,
                    op0=mybir.AluOpType.max, op1=mybir.AluOpType.add,
                    accum_out=S_d[:, sb:sb + 1])

            nc.sync.dma_start(sa_d[:], S_a[:])
            nc.sync.dma_start(sd_d[:], S_d[:])

    nc.compile()
    return nc


def _build_phase2():
    nc = bacc.Bacc("TRN2", target_bir_lowering=False, debug=False, num_devices=1)
    mean_d = nc.dram_tensor("mean", [128, NSEG], BF16, kind="ExternalInput").ap()
    w2_d = nc.dram_tensor("w2", [128, 128], BF16, kind="ExternalInput").ap()
    w3_d = nc.dram_tensor("w3", [128, 128], BF16, kind="ExternalInput").ap()
    w4_d = nc.dram_tensor("w4", [128, ODIM], BF16, kind="ExternalInput").ap()
    b2_d = nc.dram_tensor("b2", [128, 1], F32, kind="ExternalInput").ap()
    b3_d = nc.dram_tensor("b3", [128, 1], F32, kind="ExternalInput").ap()
    b4_d = nc.dram_tensor("b4", [ODIM, 1], F32, kind="ExternalInput").ap()
    out_d = nc.dram_tensor("out_t", [ODIM, NSEG], F32, kind="ExternalOutput").ap()

    with tile.TileContext(nc) as tc:
        with tc.tile_pool(name="sb", bufs=1) as pool, \
             tc.tile_pool(name="ps", bufs=2, space="PSUM") as psp:
            mean = pool.tile([128, NSEG], BF16)
            nc.sync.dma_start(mean[:], mean_d[:])
            w2 = pool.tile([128, 128], BF16)
            nc.sync.dma_start(w2[:], w2_d[:])
            w3 = pool.tile([128, 128], BF16)
            nc.sync.dma_start(w3[:], w3_d[:])
            w4 = pool.tile([128, ODIM], BF16)
            nc.sync.dma_start(w4[:], w4_d[:])
            b2 = pool.tile([128, 1], F32)
            nc.sync.dma_start(b2[:], b2_d[:])
            b3 = pool.tile([128, 1], F32)
            nc.sync.dma_start(b3[:], b3_d[:])
            b4 = pool.tile([ODIM, 1], F32)
            nc.sync.dma_start(b4[:], b4_d[:])

            hid = pool.tile([128, NSEG], BF16)
            t3 = pool.tile([128, NSEG], BF16)
            ot = pool.tile([ODIM, NSEG], F32)
            for j in range(NSEG // 512):
                sl = slice(512 * j, 512 * j + 512)
                p2 = psp.tile([128, 512], F32, tag="p")
                nc.tensor.matmul(p2[:], lhsT=w2[:], rhs=mean[:, sl],
                                 start=True, stop=True)
                nc.scalar.activation(out=hid[:, sl], in_=p2[:],
                                     func=mybir.ActivationFunctionType.Identity,
                                     bias=b2[:, 0:1])
            for j in range(NSEG // 512):
                sl = slice(512 * j, 512 * j + 512)
                p3 = psp.tile([128, 512], F32, tag="p")
                nc.tensor.matmul(p3[:], lhsT=w3[:], rhs=hid[:, sl],
                                 start=True, stop=True)
                nc.scalar.activation(out=t3[:, sl], in_=p3[:],
                                     func=mybir.ActivationFunctionType.Relu,
                                     bias=b3[:, 0:1])
            for j in range(NSEG // 512):
                sl = slice(512 * j, 512 * j + 512)
                p4f = psp.tile([128, 512], F32, tag="p")
                p4 = p4f[:ODIM, :]
                nc.tensor.matmul(p4, lhsT=w4[:], rhs=t3[:, sl],
                                 start=True, stop=True)
                nc.scalar.activation(out=ot[:, sl], in_=p4,
                                     func=mybir.ActivationFunctionType.Identity,
                                     bias=b4[:, 0:1])
            nc.sync.dma_start(out_d[:], ot[:])
    nc.compile()
    return nc


def run(inputs, ncores=NCORES, trace=False):
    x = np.asarray(inputs["x"], dtype=np.float32)
    xb = np.asarray(inputs["x_batch"])
    W1 = np.asarray(inputs["W1"], dtype=np.float32)
    b1 = np.asarray(inputs["b1"], dtype=np.float32)

    xts, meta = _host_prep(x, xb, ncores=ncores)
    n_sb, cols = meta["n_sb"], meta["cols"]

    wza = np.zeros((128, 128), dtype=np.float32)
    wza[0:64, :] = W1
    wzb = np.zeros((128, 128), dtype=np.float32)
    wzb[64:128, :] = W1
    wza = wza.astype(ml_dtypes.bfloat16)
    wzb = wzb.astype(ml_dtypes.bfloat16)
    b1c = np.ascontiguousarray(b1, np.float32).reshape(128, 1)
    nb1c = np.ascontiguousarray(-b1, np.float32).reshape(128, 1)
    in_maps = [dict(xt=xts[c], wza=wza, wzb=wzb, b1=b1c, nb1=nb1c)
               for c in range(ncores)]

    nc1 = _build_phase1(n_sb, cols, ncores=ncores)
    res1 = run_bass_kernel_spmd(nc1, in_maps, core_ids=list(range(ncores)),
                                trace=trace)

    # host: route superblock partials to segments, 8-core combine.
    # Vector-path sums are sum(max(psum,-b1)) = sum(relu(psum+b1)) - SB*b1.
    b1f = b1.astype(np.float64)
    gsums = np.zeros((NSEG, 128), dtype=np.float64)
    for c in range(ncores):
        Sa = res1.results[c]["s_act"].astype(np.float64)   # [128, n_sb]
        Sd = res1.results[c]["s_dve"].astype(np.float64)
        seg = np.array(meta["seg_a"][c], dtype=np.int64)
        if len(seg):
            np.add.at(gsums, seg, Sa.T[:len(seg)])
        seg = np.array(meta["seg_d"][c], dtype=np.int64)
        if len(seg):
            np.add.at(gsums, seg, Sd.T[:len(seg)] + SB * b1f[None, :])
    # remove the relu(b1) contribution of zero-pad columns
    gsums -= np.maximum(b1, 0.0)[None, :].astype(np.float64) * meta["padcount"][:, None]

    counts = meta["counts"]
    mean = gsums / np.maximum(counts, 1.0)[:, None]                  # [NSEG,128]

    p2_ins = [dict(
        mean=np.ascontiguousarray(mean.T.astype(ml_dtypes.bfloat16)),
        w2=np.ascontiguousarray(inputs["W2"], np.float32).astype(ml_dtypes.bfloat16),
        w3=np.ascontiguousarray(inputs["W3"], np.float32).astype(ml_dtypes.bfloat16),
        w4=np.ascontiguousarray(inputs["W4"], np.float32).astype(ml_dtypes.bfloat16),
        b2=np.ascontiguousarray(inputs["b2"], np.float32).reshape(128, 1),
        b3=np.ascontiguousarray(inputs["b3"], np.float32).reshape(128, 1),
        b4=np.ascontiguousarray(inputs["b4"], np.float32).reshape(ODIM, 1),
    )]
    nc2 = _build_phase2()
    res2 = run_bass_kernel_spmd(nc2, p2_ins, core_ids=[0], trace=trace)
    out = np.ascontiguousarray(res2.results[0]["out_t"].T).astype(np.float32)

    # segments with no nodes: reference's hid is 0 (not b2), so
    # out = relu(b3) @ W4 + b4 exactly
    empty = counts == 0
    if empty.any():
        row = (np.maximum(np.asarray(inputs["b3"], np.float64), 0.0)
               @ np.asarray(inputs["W4"], np.float64)
               + np.asarray(inputs["b4"], np.float64))
        out[empty] = row.astype(np.float32)
    return out, res1, res2


def kernel(**inputs):
    inputs = {k: np.asarray(v) for k, v in inputs.items()}
    out, _, _ = run(inputs)
    return out


if __name__ == "__main__":
    rng = np.random.default_rng(0)
    N, D, HN, B = 8 * 32 * SB, 64, 128, 64
    x = rng.standard_normal((N, D), dtype=np.float32)
    xb = np.sort(rng.integers(0, B, N).astype(np.int32))
    W1 = (rng.standard_normal((D, HN)) / 8).astype(np.float32)
    W2 = (rng.standard_normal((HN, HN)) / 11.3).astype(np.float32)
    W3 = (rng.standard_normal((HN, HN)) / 11.3).astype(np.float32)
    W4 = (rng.standard_normal((HN, ODIM)) / 11.3).astype(np.float32)
    b1 = rng.standard_normal(HN).astype(np.float32) * 0.1
    b2 = rng.standard_normal(HN).astype(np.float32) * 0.1
    b3 = rng.standard_normal(HN).astype(np.float32) * 0.1
    b4 = rng.standard_normal(ODIM).astype(np.float32) * 0.1
    ins = dict(x=x, x_batch=xb, W1=W1, b1=b1, W2=W2, b2=b2, W3=W3, b3=b3,
               W4=W4, b4=b4)
    out = kernel(**ins)

    h = np.maximum(x @ W1 + b1, 0) @ W2 + b2
    sums = np.zeros((1024, HN), dtype=np.float64)
    np.add.at(sums, xb, h.astype(np.float64))
    cnt = np.bincount(xb, minlength=1024).astype(np.float64)
    mean = sums / np.maximum(cnt, 1)[:, None]
    ref = (np.maximum(mean @ W3 + b3, 0) @ W4 + b4).astype(np.float32)
    num = np.linalg.norm(out - ref)
    den = np.linalg.norm(ref)
    print("Relative error:", num / den)
